# revision 21
# baseline (speedup 1.0000x reference)
"""Trainium2 Bass kernel: out = x @ ((W_int + offset) * scale).

Math: out[m,n] = scale[n] * ((x @ W)[m,n] + offset[n] * rowsum(x)[m]),
so the dequantized weight is never materialized: plain bf16 matmul
(W ints 0..126 are exact in bf16) plus a rank-1 epilogue fused into two
vector-engine ops per output tile.

Sharding: column-parallel — W / scale / offset / out split along N across
8 cores; x (as bf16 x^T) replicated.

Per-core kernel: the whole W shard (11 MB bf16) is cached in SBUF, its 32
blocks spread across the three DMA issuers (scalar/gpsimd early, sync joins
once the first x-tiles are through); x m-tiles (host-retiled so each load is
128 x 8KB contiguous) stream with double-buffering; PSUM accumulates over
32 K-blocks per output tile. The first two m-tiles run ko-synchronously so
the PE consumes W blocks at their arrival rate during the W-load phase.
Measured ~628 us HW exec (8 cores), rel err ~1.7e-3 vs float64 reference.
"""

import numpy as np
import ml_dtypes

M, K, N = 4096, 4096, 11008
NCORES = 8
NSH = N // NCORES  # 1376
P = 128
KO = K // P        # 32
MO = M // P        # 32
N_TILES = [(0, 512), (512, 512), (1024, 352)]

_BF16 = ml_dtypes.bfloat16

_cache = {}


def _build_nc():
    import concourse.bacc as bacc
    import concourse.mybir as mybir
    import concourse.tile as tile

    bf16 = mybir.dt.bfloat16
    f32 = mybir.dt.float32

    nc = bacc.Bacc(None, target_bir_lowering=False)
    # xt is host-retiled: xt[mo*P + p, ko*P + c] = x[mo*P + c, ko*P + p],
    # so each m-tile load is 128 partitions x 8KB fully contiguous.
    xt = nc.dram_tensor("xt", [M, K], bf16, kind="ExternalInput")
    w = nc.dram_tensor("w", [K, NSH], bf16, kind="ExternalInput")
    scaleb = nc.dram_tensor("scaleb", [P, NSH], f32, kind="ExternalInput")
    offb = nc.dram_tensor("offb", [P, NSH], f32, kind="ExternalInput")
    scol = nc.dram_tensor("scol", [P, MO], f32, kind="ExternalInput")
    out = nc.dram_tensor("out", [M, NSH], f32, kind="ExternalOutput")

    xt3 = xt.ap().rearrange("(mo p) f -> p mo f", p=P)    # [128, 32, 4096]
    w3 = w.ap().rearrange("(ko p) n -> p ko n", p=P)      # [128, 32, 1376]
    out3 = out.ap().rearrange("(mo p) n -> p mo n", p=P)  # [128, 32, 1376]

    with tile.TileContext(nc) as tc:
        with (
            tc.tile_pool(name="wpool", bufs=1) as wpool,
            tc.tile_pool(name="xpool", bufs=5) as xpool,
            tc.tile_pool(name="opool", bufs=3) as opool,
            tc.tile_pool(name="cpool", bufs=1) as cpool,
            tc.tile_pool(name="psp", bufs=8, space="PSUM") as psp,
        ):
            x_tiles = {}

            def load_x(mo):
                t = xpool.tile([P, KO, P], bf16, tag="x")
                nc.sync.dma_start(
                    t[:], xt3[:, mo, :].rearrange("p (ko c) -> p ko c", c=P)
                )
                x_tiles[mo] = t

            # First three x m-tiles up front (they run ko-synchronous with
            # the W-block arrival during the W-load phase).
            load_x(0)
            load_x(1)
            load_x(2)

            scale_sb = cpool.tile([P, NSH], f32, tag="scale")
            nc.sync.dma_start(scale_sb[:], scaleb[:])
            off_sb = cpool.tile([P, NSH], f32, tag="off")
            nc.gpsimd.dma_start(off_sb[:], offb[:])
            scol_sb = cpool.tile([P, MO], f32, tag="scol")
            nc.gpsimd.dma_start(scol_sb[:], scol[:])

            # W shard: 32 contiguous 352KB blocks spread across the DMA
            # issuers. Early blocks avoid the sync queue (busy with x0/x1)
            # so they arrive in consumption order; sync joins from ko=12.
            w_sb = []
            for ko in range(KO):
                t = wpool.tile([P, NSH], bf16, tag=f"w{ko}")
                if ko < 12:
                    eng = [nc.scalar, nc.gpsimd][ko % 2]
                else:
                    eng = [nc.scalar, nc.gpsimd, nc.sync][ko % 3]
                eng.dma_start(t[:], w3[:, ko, :])
                w_sb.append(t)

            load_x(3)
            load_x(4)

            def epilogue(mo, x_sb, ps_tiles):
                o_sb = opool.tile([P, NSH], f32, tag="o")
                for ti, (n0, nw) in enumerate(N_TILES):
                    ps = ps_tiles[ti]
                    # ps += offset[n] * s[m]   (rank-1 term, fused DVE op)
                    nc.vector.scalar_tensor_tensor(
                        ps[:, :nw],
                        off_sb[:, n0:n0 + nw],
                        scol_sb[:, mo:mo + 1],
                        ps[:, :nw],
                        mybir.AluOpType.mult,
                        mybir.AluOpType.add,
                    )
                    # out = ps * scale[n]
                    nc.vector.tensor_mul(
                        out=o_sb[:, n0:n0 + nw],
                        in0=ps[:, :nw],
                        in1=scale_sb[:, n0:n0 + nw],
                    )
                    # store per n-tile so the tail overlaps the epilogue
                    nc.scalar.dma_start(
                        out3[:, mo, n0:n0 + nw], o_sb[:, n0:n0 + nw]
                    )

            # Phase 1: m-tiles 0/1 ko-synchronous over the W blocks in their
            # MEASURED arrival order (PSUM accumulation is order-agnostic;
            # delivery is HBM-bandwidth-bound, ~1.4us/block, out of ko
            # order). m-tile 2 joins with its two 512 n-tiles from slot
            # JOIN on (2 spare PSUM banks) so PE consumption (~1.57us/
            # block) tracks delivery instead of idling.
            KO_SORTED = [0, 2, 1, 4, 3, 5, 6, 7, 8, 9, 11, 10, 13, 14,
                         16, 12, 19, 22, 17, 15, 25, 28, 18, 20, 31, 21,
                         24, 27, 23, 30, 26, 29]
            JOIN = 6
            ps_f = [
                [
                    psp.tile([P, 512], f32, tag="ps", name=f"ps_f{g}_{ti}")
                    for ti in range(len(N_TILES))
                ]
                for g in range(2)
            ]
            ps_j = [
                psp.tile([P, 512], f32, tag="ps", name=f"ps_j{ti}")
                for ti in range(2)
            ]
            for si, ko in enumerate(KO_SORTED):
                for g in range(2):
                    x_sb = x_tiles[g]
                    for ti, (n0, nw) in enumerate(N_TILES):
                        nc.tensor.matmul(
                            ps_f[g][ti][:, :nw],
                            x_sb[:, ko, :],
                            w_sb[ko][:, n0:n0 + nw],
                            start=(si == 0),
                            stop=(si == KO - 1),
                        )
                if si >= JOIN:
                    for ti in range(2):
                        n0, nw = N_TILES[ti]
                        nc.tensor.matmul(
                            ps_j[ti][:, :nw],
                            x_tiles[2][:, ko, :],
                            w_sb[ko][:, n0:n0 + nw],
                            start=(si == JOIN),
                            stop=False,
                        )
            for g in range(2):
                epilogue(g, x_tiles.pop(g), ps_f[g])

            # m-tile 2: catch up the blocks it skipped before JOIN, then
            # its third n-tile (all W is cached by now).
            for j, ko in enumerate(KO_SORTED[:JOIN]):
                for ti in range(2):
                    n0, nw = N_TILES[ti]
                    nc.tensor.matmul(
                        ps_j[ti][:, :nw],
                        x_tiles[2][:, ko, :],
                        w_sb[ko][:, n0:n0 + nw],
                        start=False,
                        stop=(j == JOIN - 1),
                    )
            n0_2, nw_2 = N_TILES[2]
            ps_2 = psp.tile([P, 512], f32, tag="ps", name="ps_m2t2")
            for ko in range(KO):
                nc.tensor.matmul(
                    ps_2[:, :nw_2],
                    x_tiles[2][:, ko, :],
                    w_sb[ko][:, n0_2:n0_2 + nw_2],
                    start=(ko == 0),
                    stop=(ko == KO - 1),
                )
            epilogue(2, x_tiles.pop(2), [ps_j[0], ps_j[1], ps_2])

            # Phase 2: remaining m-tiles, streaming.
            for mo in range(3, MO):
                if mo + 2 < MO:
                    load_x(mo + 2)
                x_sb = x_tiles.pop(mo)
                ps_tiles = []
                for n0, nw in N_TILES:
                    ps = psp.tile([P, 512], f32, tag="ps")
                    for ko in range(KO):
                        nc.tensor.matmul(
                            ps[:, :nw],
                            x_sb[:, ko, :],
                            w_sb[ko][:, n0:n0 + nw],
                            start=(ko == 0),
                            stop=(ko == KO - 1),
                        )
                    ps_tiles.append(ps)
                epilogue(mo, x_sb, ps_tiles)
    nc.compile()
    return nc


def _get_nc():
    if "nc" not in _cache:
        _cache["nc"] = _build_nc()
    return _cache["nc"]


def _prep_inputs(x, weight, antiquant_scale, antiquant_offset):
    x = np.asarray(x, dtype=np.float32)
    weight = np.asarray(weight)
    antiquant_scale = np.asarray(antiquant_scale, dtype=np.float32)
    antiquant_offset = np.asarray(antiquant_offset, dtype=np.float32)

    # Blocked transpose: xt[mo, p, ko, c] = x[mo*P + c, ko*P + p]
    xt = np.ascontiguousarray(
        x.reshape(MO, P, KO, P).transpose(0, 3, 2, 1).astype(_BF16)
    ).reshape(M, K)
    s = x.sum(axis=1, dtype=np.float32)                      # [M]
    scol = np.ascontiguousarray(s.reshape(MO, P).T)          # [P, MO]

    in_maps = []
    for c in range(NCORES):
        sl = slice(c * NSH, (c + 1) * NSH)
        wc = np.ascontiguousarray(weight[:, sl].astype(_BF16))
        scaleb = np.ascontiguousarray(
            np.broadcast_to(antiquant_scale[sl][None, :], (P, NSH))
        )
        offb = np.ascontiguousarray(
            np.broadcast_to(antiquant_offset[sl][None, :], (P, NSH))
        )
        in_maps.append(
            {"xt": xt, "w": wc, "scaleb": scaleb, "offb": offb, "scol": scol}
        )
    return in_maps


def kernel(x, weight, antiquant_scale, antiquant_offset, _trace=False):
    from concourse.bass_utils import run_bass_kernel_spmd

    nc = _get_nc()
    in_maps = _prep_inputs(x, weight, antiquant_scale, antiquant_offset)
    res = run_bass_kernel_spmd(
        nc, in_maps, core_ids=list(range(NCORES)), trace=_trace
    )
    out = np.concatenate([res.results[c]["out"] for c in range(NCORES)], axis=1)
    if _trace:
        _cache["last_result"] = res
    return out


# revision 23
# speedup vs baseline: 1.0157x; 1.0157x over previous
"""Trainium2 Bass kernel: out = x @ ((W_int + offset) * scale).

Math: out[m,n] = scale[n] * ((x @ W)[m,n] + offset[n] * rowsum(x)[m]),
so the dequantized weight is never materialized: plain bf16 matmul
(W ints 0..126 are exact in bf16) plus a rank-1 epilogue fused into two
vector-engine ops per output tile.

Sharding: column-parallel — W / scale / offset / out split along N across
8 cores; x (as bf16 x^T) replicated.

Per-core kernel: the whole W shard (11 MB bf16) is cached in SBUF, its 32
blocks spread across the three DMA issuers (scalar/gpsimd early, sync joins
once the first x-tiles are through); x m-tiles (host-retiled so each load is
128 x 8KB contiguous) stream with double-buffering; PSUM accumulates over
32 K-blocks per output tile. The first two m-tiles run ko-synchronously so
the PE consumes W blocks at their arrival rate during the W-load phase.
Measured ~628 us HW exec (8 cores), rel err ~1.7e-3 vs float64 reference.
"""

import numpy as np
import ml_dtypes

M, K, N = 4096, 4096, 11008
NCORES = 8
NSH = N // NCORES  # 1376
P = 128
KO = K // P        # 32
MO = M // P        # 32
N_TILES = [(0, 512), (512, 512), (1024, 352)]

_BF16 = ml_dtypes.bfloat16

_cache = {}


def _build_nc():
    import concourse.bacc as bacc
    import concourse.mybir as mybir
    import concourse.tile as tile

    bf16 = mybir.dt.bfloat16
    f32 = mybir.dt.float32

    nc = bacc.Bacc(None, target_bir_lowering=False)
    # xt is host-retiled: xt[mo*P + p, ko*P + c] = x[mo*P + c, ko*P + p],
    # so each m-tile load is 128 partitions x 8KB fully contiguous.
    xt = nc.dram_tensor("xt", [M, K], bf16, kind="ExternalInput")
    w = nc.dram_tensor("w", [K, NSH], bf16, kind="ExternalInput")
    scaleb = nc.dram_tensor("scaleb", [P, NSH], f32, kind="ExternalInput")
    offb = nc.dram_tensor("offb", [P, NSH], f32, kind="ExternalInput")
    scol = nc.dram_tensor("scol", [P, MO], f32, kind="ExternalInput")
    out = nc.dram_tensor("out", [M, NSH], f32, kind="ExternalOutput")

    xt3 = xt.ap().rearrange("(mo p) f -> p mo f", p=P)    # [128, 32, 4096]
    w3 = w.ap().rearrange("(ko p) n -> p ko n", p=P)      # [128, 32, 1376]
    out3 = out.ap().rearrange("(mo p) n -> p mo n", p=P)  # [128, 32, 1376]

    with tile.TileContext(nc) as tc:
        with (
            tc.tile_pool(name="wpool", bufs=1) as wpool,
            tc.tile_pool(name="xpool", bufs=5) as xpool,
            tc.tile_pool(name="opool", bufs=3) as opool,
            tc.tile_pool(name="cpool", bufs=1) as cpool,
            tc.tile_pool(name="psp", bufs=8, space="PSUM") as psp,
        ):
            x_tiles = {}

            def load_x(mo):
                t = xpool.tile([P, KO, P], bf16, tag="x")
                nc.sync.dma_start(
                    t[:], xt3[:, mo, :].rearrange("p (ko c) -> p ko c", c=P)
                )
                x_tiles[mo] = t

            # First three x m-tiles up front (they run ko-synchronous with
            # the W-block arrival during the W-load phase).
            load_x(0)
            load_x(1)
            load_x(2)

            # W shard: 32 contiguous 352KB blocks spread across the DMA
            # issuers. Early blocks avoid the sync queue (busy with x0-x2)
            # so they arrive early; sync joins from ko=12. Epilogue
            # constants go to queue tails (not needed until ~+55us).
            w_sb = []
            for ko in range(KO):
                t = wpool.tile([P, NSH], bf16, tag=f"w{ko}")
                if ko < 12:
                    eng = [nc.scalar, nc.gpsimd][ko % 2]
                else:
                    eng = [nc.scalar, nc.gpsimd, nc.sync][ko % 3]
                eng.dma_start(t[:], w3[:, ko, :])
                w_sb.append(t)

            scale_sb = cpool.tile([P, NSH], f32, tag="scale")
            nc.sync.dma_start(scale_sb[:], scaleb[:])
            off_sb = cpool.tile([P, NSH], f32, tag="off")
            nc.gpsimd.dma_start(off_sb[:], offb[:])
            scol_sb = cpool.tile([P, MO], f32, tag="scol")
            nc.gpsimd.dma_start(scol_sb[:], scol[:])

            load_x(3)
            load_x(4)

            def epilogue(mo, x_sb, ps_tiles):
                o_sb = opool.tile([P, NSH], f32, tag="o")
                for ti, (n0, nw) in enumerate(N_TILES):
                    ps = ps_tiles[ti]
                    # ps += offset[n] * s[m]   (rank-1 term, fused DVE op)
                    nc.vector.scalar_tensor_tensor(
                        ps[:, :nw],
                        off_sb[:, n0:n0 + nw],
                        scol_sb[:, mo:mo + 1],
                        ps[:, :nw],
                        mybir.AluOpType.mult,
                        mybir.AluOpType.add,
                    )
                    # out = ps * scale[n]
                    nc.vector.tensor_mul(
                        out=o_sb[:, n0:n0 + nw],
                        in0=ps[:, :nw],
                        in1=scale_sb[:, n0:n0 + nw],
                    )
                    # store per n-tile so the tail overlaps the epilogue
                    nc.scalar.dma_start(
                        out3[:, mo, n0:n0 + nw], o_sb[:, n0:n0 + nw]
                    )

            # Phase 1: W delivery is HBM-bandwidth-bound (~1.4us/block,
            # out of ko order), so three m-tile groups consume the block
            # stream in approximate ARRIVAL order (PSUM accumulation is
            # order-agnostic) with staggered lags: m-tile 0 tracks the
            # frontier, m-tile 1 runs LAG1 slots behind, m-tile 2 (its two
            # 512 n-tiles; 2 spare PSUM banks) LAG2 behind. The lag keeps
            # already-arrived blocks available whenever a fresh block is
            # late, so the PE stays busy through the whole load phase.
            KO_SORTED = [0, 2, 1, 4, 3, 5, 6, 7, 8, 9, 11, 10, 13, 14,
                         16, 12, 19, 22, 17, 15, 25, 28, 18, 20, 31, 21,
                         24, 27, 23, 30, 26, 29]
            LAG1, LAG2 = 3, 8
            ps_f = [
                [
                    psp.tile([P, 512], f32, tag="ps", name=f"ps_f{g}_{ti}")
                    for ti in range(len(N_TILES))
                ]
                for g in range(2)
            ]
            ps_j = [
                psp.tile([P, 512], f32, tag="ps", name=f"ps_j{ti}")
                for ti in range(2)
            ]

            def fused_block(g, idx):
                ko = KO_SORTED[idx]
                ntiles = N_TILES if g < 2 else N_TILES[:2]
                pss = ps_f[g] if g < 2 else ps_j
                for ti, (n0, nw) in enumerate(ntiles):
                    nc.tensor.matmul(
                        pss[ti][:, :nw],
                        x_tiles[g][:, ko, :],
                        w_sb[ko][:, n0:n0 + nw],
                        start=(idx == 0),
                        stop=(idx == KO - 1),
                    )

            for si in range(KO + LAG2):
                for g, lag in ((0, 0), (1, LAG1), (2, LAG2)):
                    idx = si - lag
                    if 0 <= idx < KO:
                        fused_block(g, idx)
            for g in range(2):
                epilogue(g, x_tiles.pop(g), ps_f[g])

            # m-tile 2's third n-tile (all W cached by now).
            n0_2, nw_2 = N_TILES[2]
            ps_2 = psp.tile([P, 512], f32, tag="ps", name="ps_m2t2")
            for ko in range(KO):
                nc.tensor.matmul(
                    ps_2[:, :nw_2],
                    x_tiles[2][:, ko, :],
                    w_sb[ko][:, n0_2:n0_2 + nw_2],
                    start=(ko == 0),
                    stop=(ko == KO - 1),
                )
            epilogue(2, x_tiles.pop(2), [ps_j[0], ps_j[1], ps_2])

            # Phase 2: remaining m-tiles, streaming.
            for mo in range(3, MO):
                if mo + 2 < MO:
                    load_x(mo + 2)
                x_sb = x_tiles.pop(mo)
                ps_tiles = []
                for n0, nw in N_TILES:
                    ps = psp.tile([P, 512], f32, tag="ps")
                    for ko in range(KO):
                        nc.tensor.matmul(
                            ps[:, :nw],
                            x_sb[:, ko, :],
                            w_sb[ko][:, n0:n0 + nw],
                            start=(ko == 0),
                            stop=(ko == KO - 1),
                        )
                    ps_tiles.append(ps)
                epilogue(mo, x_sb, ps_tiles)
    nc.compile()
    return nc


def _get_nc():
    if "nc" not in _cache:
        _cache["nc"] = _build_nc()
    return _cache["nc"]


def _prep_inputs(x, weight, antiquant_scale, antiquant_offset):
    x = np.asarray(x, dtype=np.float32)
    weight = np.asarray(weight)
    antiquant_scale = np.asarray(antiquant_scale, dtype=np.float32)
    antiquant_offset = np.asarray(antiquant_offset, dtype=np.float32)

    # Blocked transpose: xt[mo, p, ko, c] = x[mo*P + c, ko*P + p]
    xt = np.ascontiguousarray(
        x.reshape(MO, P, KO, P).transpose(0, 3, 2, 1).astype(_BF16)
    ).reshape(M, K)
    s = x.sum(axis=1, dtype=np.float32)                      # [M]
    scol = np.ascontiguousarray(s.reshape(MO, P).T)          # [P, MO]

    in_maps = []
    for c in range(NCORES):
        sl = slice(c * NSH, (c + 1) * NSH)
        wc = np.ascontiguousarray(weight[:, sl].astype(_BF16))
        scaleb = np.ascontiguousarray(
            np.broadcast_to(antiquant_scale[sl][None, :], (P, NSH))
        )
        offb = np.ascontiguousarray(
            np.broadcast_to(antiquant_offset[sl][None, :], (P, NSH))
        )
        in_maps.append(
            {"xt": xt, "w": wc, "scaleb": scaleb, "offb": offb, "scol": scol}
        )
    return in_maps


def kernel(x, weight, antiquant_scale, antiquant_offset, _trace=False):
    from concourse.bass_utils import run_bass_kernel_spmd

    nc = _get_nc()
    in_maps = _prep_inputs(x, weight, antiquant_scale, antiquant_offset)
    res = run_bass_kernel_spmd(
        nc, in_maps, core_ids=list(range(NCORES)), trace=_trace
    )
    out = np.concatenate([res.results[c]["out"] for c in range(NCORES)], axis=1)
    if _trace:
        _cache["last_result"] = res
    return out
